# revision 17
# baseline (speedup 1.0000x reference)
"""Trainium2 Bass kernel for nn_Block_59983513256170 (dense transformer block).

Sharding: 8 cores = (batch 4) x (sequence halves 2). Each core computes the
block for 512 query tokens of one batch element, redundantly computing
LN1+quant and K/V over that batch element's full 1024-token sequence so no
cross-core communication is needed. Host rotates each core's token order so
its own 512 tokens are always rows [0:512] (attention is permutation
invariant over keys), letting all cores run one identical SPMD program.

Precision: attention branch (qkv/scores/AV/proj) runs on fp32r matmuls
(~12 mantissa bits at bf16 speed) because its output feeds the second
block-FP quantization, where noise flips quantization bins. The MLP runs in
bf16 (its error does not cross a quantization boundary). LN, BFP quant,
softmax arithmetic and residuals are fp32. The BFP quantization uses an
exact bit-trick: scale = pow2(amax) via mantissa masking, round-to-nearest-
even via the +/- 1.5*2^23*scale addition trick, then clamp to +/-7*scale.
"""

import sys

sys.path.insert(0, "/opt/trn_rl_repo")

import numpy as np
import ml_dtypes

import concourse.bass as bass
import concourse.bacc as bacc
import concourse.tile as tile
import concourse.mybir as mybir
from concourse import bass_utils
from concourse.masks import make_identity

F32 = mybir.dt.float32
F32R = mybir.dt.float32r
BF16 = mybir.dt.bfloat16
AF = mybir.ActivationFunctionType
OP = mybir.AluOpType

D = 1024
H = 16
DH = 64
DFF = 4096
LN_EPS = 1e-6


def bcast16(t):
    """View a [128, nb] tile as [128, nb, 16] with the last dim broadcast."""
    ap = [list(x) for x in t.ap]
    return bass.AP(tensor=t.tensor, offset=t.offset, ap=ap + [[0, 16]])


def build_nc(Tq, Tkv, apply_gb=True):
    """Build the per-core Bass program. Tq = own query tokens, Tkv = full
    sequence tokens of this core's batch element (own tokens first)."""
    nq = Tq // 128   # query token tiles
    nk = Tkv // 128  # kv token tiles
    nc = bacc.Bacc("TRN2", target_bir_lowering=False, debug=False)

    x_d = nc.dram_tensor("x", [Tkv, D], F32, kind="ExternalInput").ap()
    wqkv_d = nc.dram_tensor("w_qkv", [D, 3 * D], F32R, kind="ExternalInput").ap()
    wproj_d = nc.dram_tensor("w_proj", [D, D], F32R, kind="ExternalInput").ap()
    bproj_d = nc.dram_tensor("b_proj", [D], F32, kind="ExternalInput").ap()
    wfc1_d = nc.dram_tensor("w_fc1", [D, DFF], BF16, kind="ExternalInput").ap()
    bfc1_d = nc.dram_tensor("b_fc1", [DFF], F32, kind="ExternalInput").ap()
    wfc2_d = nc.dram_tensor("w_fc2", [DFF, D], BF16, kind="ExternalInput").ap()
    bfc2_d = nc.dram_tensor("b_fc2", [D], F32, kind="ExternalInput").ap()
    g1_d = nc.dram_tensor("ln1_g", [D], F32, kind="ExternalInput").ap()
    b1_d = nc.dram_tensor("ln1_b", [D], F32, kind="ExternalInput").ap()
    g2_d = nc.dram_tensor("ln2_g", [D], F32, kind="ExternalInput").ap()
    b2_d = nc.dram_tensor("ln2_b", [D], F32, kind="ExternalInput").ap()
    out_d = nc.dram_tensor("out", [Tq, D], F32, kind="ExternalOutput").ap()

    def vec_bcast(pool, dram_vec, name, dtype=F32):
        """DRAM [D] vector -> SBUF [128, D] broadcast tile."""
        t = pool.tile([128, dram_vec.shape[0]], dtype, name=name)
        src = bass.AP(tensor=dram_vec.tensor, offset=dram_vec.offset,
                      ap=[[0, 128]] + [list(x) for x in dram_vec.ap])
        nc.sync.dma_start(out=t, in_=src)
        return t

    with tile.TileContext(nc) as tc:
        _cms = {}

        def open_pool(name, bufs, space="SBUF"):
            cm = tc.tile_pool(name=name, bufs=bufs, space=space)
            _cms[name] = cm
            return cm.__enter__()

        def close_pool(name):
            _cms.pop(name).__exit__(None, None, None)

        consts = open_pool("consts", 1)
        psum = open_pool("psum", 7, space="PSUM")
        dummy_ps = open_pool("dummy_ps", 1, space="PSUM")
        resid = open_pool("resid", 1)
        small = open_pool("small", 4)
        h2Tp = open_pool("h2Tp", 1)
        h2p = open_pool("h2p", 2)
        attn_big = open_pool("attn_big", 1)

        ident = consts.tile([128, 128], BF16, name="ident")
        make_identity(nc, ident)
        eps_t = consts.tile([128, 1], F32, name="eps")
        nc.vector.memset(eps_t, LN_EPS)
        if apply_gb:
            g1b = vec_bcast(consts, g1_d, "g1b")
            b1b = vec_bcast(consts, b1_d, "b1b")
            g2b = vec_bcast(consts, g2_d, "g2b")
            b2b = vec_bcast(consts, b2_d, "b2b")
        else:
            g1b = b1b = g2b = b2b = None
        bpb = vec_bcast(consts, bproj_d, "bpb")
        bf2b = vec_bcast(consts, bfc2_d, "bf2b")
        # b_fc1 as per-partition bias columns: [128, 32], [p, c] = b_fc1[c*128+p]
        bfc1_sb = consts.tile([128, DFF // 128], F32, name="bfc1")
        nc.sync.dma_start(out=bfc1_sb, in_=bfc1_d.rearrange("(c p) -> p c", p=128))

        # persistent across attention: packed qT/kT/v65/o and residual stream
        qT = attn_big.tile([128, 8, Tq], F32R, name="qT")       # q feature-major
        kT = attn_big.tile([128, 8, Tkv], F32R, name="kT")      # k feature-major
        v65 = attn_big.tile([128, H, nk, 65], F32R, name="v65")  # v token-major + ones col
        o_p = attn_big.tile([128, 8, Tq], F32R, name="o_p")     # normalized attn out
        x2 = resid.tile([128, nq, D], F32, name="x2")        # x + b_proj, then attn residual

        idf32 = consts.tile([128, 128], F32, name="idf32")
        nc.vector.memset(idf32, 0.0)

        def warm_f32(dep):
            """Small PE touch dependent on a fp32 tile (as rhs): defeats the
            HAM idle-detector during DVE-bound stretches."""
            k = dep.shape[0]
            dp = dummy_ps.tile([128, 64], F32, name="dummy")
            nc.tensor.matmul(dp, idf32[0:k, :], dep[:, 0:64], start=True, stop=True)

        def warm_bf16(dep):
            dp = dummy_ps.tile([128, 64], F32, name="dummy")
            nc.tensor.matmul(dp, ident, dep[:, 0:64], start=True, stop=True)

        def layernorm_quant(xt, g_b, b_b, hpool, hname):
            """token-major [128, D] fp32 -> LN -> BFP quant -> bf16 tile."""
            st = small.tile([128, 2, 6], F32, name="bnst")
            nc.vector.bn_stats(out=st[:, 0, :], in_=xt[:, 0:512])
            nc.vector.bn_stats(out=st[:, 1, :], in_=xt[:, 512:1024])
            mv = small.tile([128, 2], F32, name="bnmv")
            nc.vector.bn_aggr(out=mv, in_=st)
            rs = small.tile([128, 1], F32, name="rs")
            nc.scalar.activation(rs, mv[:, 1:2], AF.Sqrt, bias=eps_t, scale=1.0)
            rr = small.tile([128, 1], F32, name="rr")
            nc.vector.reciprocal(rr, rs)
            ht = hpool.tile([128, D], F32, name=hname + "_f")
            nc.vector.tensor_scalar(out=ht, in0=xt, scalar1=mv[:, 0:1], scalar2=rr,
                                    op0=OP.subtract, op1=OP.mult)
            if g_b is not None:
                nc.gpsimd.tensor_tensor(out=ht, in0=ht, in1=g_b, op=OP.mult)
                nc.gpsimd.tensor_tensor(out=ht, in0=ht, in1=b_b, op=OP.add)
            # BFP quant: blocks of 16 along features
            nb = D // 16
            amax = small.tile([128, nb], F32, name="amax")
            nc.vector.tensor_reduce(amax, ht.rearrange("p (b k) -> p b k", k=16),
                                    axis=mybir.AxisListType.X, op=OP.max,
                                    apply_absolute_value=True)
            nc.vector.tensor_scalar(out=amax.bitcast(mybir.dt.uint32),
                                    in0=amax.bitcast(mybir.dt.uint32),
                                    scalar1=0xFF800000, scalar2=None,
                                    op0=OP.bitwise_and)
            cc = small.tile([128, nb], F32, name="cc")
            nc.vector.tensor_scalar_mul(cc, amax, float(1.5 * 2 ** 20))
            warm_f32(cc)
            hi = small.tile([128, nb], F32, name="hi")
            nc.vector.tensor_scalar_mul(hi, amax, 7.0 / 8.0)
            lo = small.tile([128, nb], F32, name="lo")
            nc.vector.tensor_scalar_mul(lo, amax, -7.0 / 8.0)
            h3 = ht.rearrange("p (b k) -> p b k", k=16)
            nc.vector.tensor_tensor(out=h3, in0=h3, in1=bcast16(cc), op=OP.add)
            nc.vector.tensor_tensor(out=h3, in0=h3, in1=bcast16(cc), op=OP.subtract)
            nc.vector.tensor_tensor(out=h3, in0=h3, in1=bcast16(hi), op=OP.min)
            hq = hpool.tile([128, D], BF16, name=hname)
            nc.vector.tensor_tensor(out=hq.rearrange("p (b k) -> p b k", k=16),
                                    in0=h3, in1=bcast16(lo), op=OP.max)
            warm_bf16(hq)
            return hq

        # ---------------- phase 1+2: LN1 + quant + transpose ----------------
        h1fmp = open_pool("h1fmp", 1)
        h1p = open_pool("h1p", 2)
        h1_fm = h1fmp.tile([128, 8, Tkv], F32R, name="h1fm")  # feature-major quantized
        with nc.named_scope("ln1_quant"):
            for tt in range(nk):
                xt = h1p.tile([128, D], F32, name="xt")
                nc.sync.dma_start(out=xt, in_=x_d[tt * 128:(tt + 1) * 128, :])
                hq = layernorm_quant(xt, g1b, b1b, h1p, "h1tok")
                for dd in range(8):
                    pst = psum.tile([128, 128], BF16, name="ps")
                    nc.tensor.transpose(pst, hq[:, dd * 128:(dd + 1) * 128], ident)
                    nc.scalar.copy(h1_fm[:, dd, tt * 128:(tt + 1) * 128], pst)
        close_pool("h1p")

        # ---------------- phase 3a: q, k (feature-major) ----------------
        # q: w cols [0, 1024)  -> qT ; k: w cols [1024, 2048) -> kT
        wkp = open_pool("wkp", 12)
        with nc.named_scope("qk_mm"):
            for blk in range(8):  # 8 blocks of 2 e-chunks (4 q blocks, 4 k blocks)
                is_q = blk < 4
                toks = Tq if is_q else Tkv
                ntc = toks // 512 if toks >= 512 else 1
                ntok = min(toks, 512)
                col0 = blk * 256 if is_q else 1024 + (blk - 4) * 256
                pss = [psum.tile([128, ntok], F32, name="ps") for _ in range(2 * ntc)]
                for d in range(8):
                    wt = wkp.tile([128, 256], F32R, name="wqk")
                    eng = nc.sync if d % 2 == 0 else nc.gpsimd
                    eng.dma_start(out=wt,
                                  in_=wqkv_d[d * 128:(d + 1) * 128, col0:col0 + 256])
                    for e2 in range(2):
                        for th in range(ntc):
                            nc.tensor.matmul(pss[e2 * ntc + th],
                                             wt[:, e2 * 128:(e2 + 1) * 128],
                                             h1_fm[:, d, th * 512:th * 512 + ntok],
                                             start=(d == 0), stop=(d == 7))
                dst = qT if is_q else kT
                ec0 = blk * 2 if is_q else (blk - 4) * 2
                for e2 in range(2):
                    for th in range(ntc):
                        nc.scalar.copy(
                            dst[:, ec0 + e2, th * 512:th * 512 + ntok],
                            pss[e2 * ntc + th])

        close_pool("wkp")

        # ---------------- phase 3b: v (token-major, + ones col) ----------------
        with nc.named_scope("v_mm"):
            for vc in range(2):
                wvp = open_pool("wvp", 1)
                wv = wvp.tile([128, 8, 512], F32R, name="wv")
                for d in range(8):
                    eng = nc.sync if d % 2 == 0 else nc.gpsimd
                    eng.dma_start(
                        out=wv[:, d, :],
                        in_=wqkv_d[d * 128:(d + 1) * 128,
                                   2048 + vc * 512:2048 + (vc + 1) * 512])
                for tch in range(nk):
                    ps = psum.tile([128, 512], F32, name="ps")
                    for d in range(8):
                        nc.tensor.matmul(ps, h1_fm[:, d, tch * 128:(tch + 1) * 128],
                                         wv[:, d, :], start=(d == 0), stop=(d == 7))
                    for hh in range(8):
                        head = vc * 8 + hh
                        nc.scalar.copy(v65[:, head, tch, 0:64],
                                       ps[:, hh * 64:(hh + 1) * 64])
                        nc.gpsimd.memset(v65[:, head, tch, 64:65].bitcast(F32), 1.0)
                close_pool("wvp")
        close_pool("h1fmp")

        # ---------------- phase 4: attention per head pair ----------------
        atp = open_pool("atp", 4)
        onp = open_pool("onp", 2)
        wpp = open_pool("wpp", 1)
        wproj_sb = wpp.tile([128, 8, D], F32R, name="wproj")
        for d in range(8):
            eng = nc.sync if d % 2 == 0 else nc.gpsimd
            eng.dma_start(out=wproj_sb[:, d, :],
                          in_=wproj_d[d * 128:(d + 1) * 128, :])
        # x + b_proj staged into x2 while attention runs
        for tcq in range(nq):
            nc.sync.dma_start(out=x2[:, tcq, :], in_=x_d[tcq * 128:(tcq + 1) * 128, :])
            nc.vector.tensor_tensor(out=x2[:, tcq, :], in0=x2[:, tcq, :], in1=bpb,
                                    op=OP.add)

        with nc.named_scope("attn"):
            for j in range(8):  # head pair (2j, 2j+1)
                ps_o = [psum.tile([65, Tq], F32, name="ps") for _ in range(2)]
                for kc in range(nk):
                    for ab in range(2):
                        ps_s = psum.tile([128, Tq], F32, name="ps")
                        nc.tensor.matmul(
                            ps_s,
                            kT[ab * 64:(ab + 1) * 64, j, kc * 128:(kc + 1) * 128],
                            qT[ab * 64:(ab + 1) * 64, j, :],
                            start=True, stop=True)
                        ee = atp.tile([128, Tq], F32R, name="expT")
                        nc.scalar.activation(ee, ps_s, AF.Exp, scale=0.125)
                        nc.tensor.matmul(ps_o[ab], v65[:, 2 * j + ab, kc, :], ee,
                                         start=(kc == 0), stop=(kc == nk - 1))
                for ab in range(2):
                    osb = onp.tile([65, Tq], F32, name="osb")
                    nc.scalar.copy(osb, ps_o[ab])
                    if ab == 0:
                        warm_f32(osb[0:64, 0:64])
                    row = onp.tile([1, Tq], F32, name="row")
                    nc.sync.dma_start(out=row, in_=osb[64:65, :])
                    rrow = onp.tile([1, Tq], F32, name="rrow")
                    nc.vector.reciprocal_approx_fast(rrow, row)
                    r64 = onp.tile([64, Tq], F32, name="r64")
                    nc.gpsimd.partition_broadcast(r64, rrow)
                    if ab == 0:
                        nc.vector.tensor_tensor(out=o_p[0:64, j, :],
                                                in0=osb[0:64, :], in1=r64,
                                                op=OP.mult)
                    else:
                        ob = onp.tile([64, Tq], F32R, name="ob")
                        nc.vector.tensor_tensor(out=ob, in0=osb[0:64, :],
                                                in1=r64, op=OP.mult)
                        nc.sync.dma_start(out=o_p[64:128, j, :], in_=ob)

        # ---------------- phase 5: proj + residual ----------------
        h2T = h2Tp.tile([128, 8, Tq], BF16, name="h2T")
        with nc.named_scope("proj_ln2"):
            for tcq in range(nq):
                for nn in range(2):
                    ps = psum.tile([128, 512], F32, name="ps")
                    for j in range(8):
                        nc.tensor.matmul(ps, o_p[:, j, tcq * 128:(tcq + 1) * 128],
                                         wproj_sb[:, j, nn * 512:(nn + 1) * 512],
                                         start=(j == 0), stop=(j == 7))
                    nc.vector.tensor_tensor(out=x2[:, tcq, nn * 512:(nn + 1) * 512],
                                            in0=ps,
                                            in1=x2[:, tcq, nn * 512:(nn + 1) * 512],
                                            op=OP.add)
                # LN2 + quant + transpose for this token tile right away
                hq = layernorm_quant(x2[:, tcq, :], g2b, b2b, h2p, "h2tok")
                for dd in range(8):
                    pst = psum.tile([128, 128], BF16, name="ps")
                    nc.tensor.transpose(pst, hq[:, dd * 128:(dd + 1) * 128], ident)
                    nc.scalar.copy(h2T[:, dd, tcq * 128:(tcq + 1) * 128], pst)
                # x2 += b_fc2 after LN2 consumed it (final residual base)
                nc.vector.tensor_tensor(out=x2[:, tcq, :], in0=x2[:, tcq, :],
                                        in1=bf2b, op=OP.add)
        close_pool("wpp")
        close_pool("onp")
        close_pool("atp")
        close_pool("attn_big")

        # ---------------- phase 7 prep ----------------
        mlp = open_pool("mlp", 1)
        mT = mlp.tile([128, DFF // 128, Tq], BF16, name="mT")
        wfc2_sb = mlp.tile([128, DFF // 128, D], BF16, name="wfc2")
        for g in range(8):
            eng = nc.sync if g % 2 == 0 else nc.gpsimd
            eng.dma_start(
                out=wfc2_sb[:, g * 4:(g + 1) * 4, :],
                in_=wfc2_d.rearrange("(c p) n -> p c n", p=128)[:, g * 4:(g + 1) * 4, :])

        # ---------------- phase 7: fc1 + gelu (feature-major m) ----------------
        wf1p = open_pool("wf1p", 6)
        wfc1_r = wfc1_d.rearrange("(c p) n -> p c n", p=128)
        with nc.named_scope("fc1"):
            for hc in range(DFF // 128):
                wt = wf1p.tile([128, 8, 128], BF16, name="wfc1")
                eng = nc.sync if hc % 2 == 0 else nc.gpsimd
                eng.dma_start(out=wt, in_=wfc1_r[:, :, hc * 128:(hc + 1) * 128])
                ps = psum.tile([128, Tq], F32, name="ps")
                for d in range(8):
                    nc.tensor.matmul(ps, wt[:, d, :], h2T[:, d, :],
                                     start=(d == 0), stop=(d == 7))
                nc.scalar.activation(mT[:, hc, :], ps, AF.Gelu,
                                     bias=bfc1_sb[:, hc:hc + 1], scale=1.0)

        # ---------------- phase 8: fc2 + residual -> out ----------------
        outp = open_pool("outp", 2)
        with nc.named_scope("fc2"):
            for tcq in range(nq):
                ot = outp.tile([128, D], F32, name="ot")
                for nn in range(2):
                    ps = psum.tile([128, 512], F32, name="ps")
                    for hc in range(DFF // 128):
                        nc.tensor.matmul(ps, mT[:, hc, tcq * 128:(tcq + 1) * 128],
                                         wfc2_sb[:, hc, nn * 512:(nn + 1) * 512],
                                         start=(hc == 0), stop=(hc == DFF // 128 - 1))
                    nc.vector.tensor_tensor(out=ot[:, nn * 512:(nn + 1) * 512],
                                            in0=ps,
                                            in1=x2[:, tcq, nn * 512:(nn + 1) * 512],
                                            op=OP.add)
                nc.sync.dma_start(out=out_d[tcq * 128:(tcq + 1) * 128, :], in_=ot)

        close_pool("outp")
        close_pool("wf1p")
        close_pool("mlp")
        close_pool("h2p")
        close_pool("h2Tp")
        close_pool("small")
        close_pool("resid")
        close_pool("dummy_ps")
        close_pool("psum")
        close_pool("consts")

    nc.finalize()
    return nc


_NC_CACHE = {}


def _get_nc(Tq, Tkv, apply_gb=True):
    key = (Tq, Tkv, apply_gb)
    if key not in _NC_CACHE:
        _NC_CACHE[key] = build_nc(Tq, Tkv, apply_gb)
    return _NC_CACHE[key]


def make_in_maps(x, ln1_g, ln1_b, ln2_g, ln2_b, w_qkv, w_proj, b_proj,
                 w_fc1, b_fc1, w_fc2, b_fc2, n_cores=8):
    x = np.asarray(x, np.float32)
    B, S, _ = x.shape
    half = S // 2
    shared = {
        "w_qkv": np.ascontiguousarray(np.asarray(w_qkv, np.float32)),
        "w_proj": np.ascontiguousarray(np.asarray(w_proj, np.float32)),
        "b_proj": np.asarray(b_proj, np.float32),
        "w_fc1": np.ascontiguousarray(np.asarray(w_fc1, np.float32).astype(ml_dtypes.bfloat16)),
        "b_fc1": np.asarray(b_fc1, np.float32),
        "w_fc2": np.ascontiguousarray(np.asarray(w_fc2, np.float32).astype(ml_dtypes.bfloat16)),
        "b_fc2": np.asarray(b_fc2, np.float32),
        "ln1_g": np.asarray(ln1_g, np.float32),
        "ln1_b": np.asarray(ln1_b, np.float32),
        "ln2_g": np.asarray(ln2_g, np.float32),
        "ln2_b": np.asarray(ln2_b, np.float32),
    }
    in_maps = []
    for c in range(n_cores):
        b, h = c // 2, c % 2
        xr = np.concatenate([x[b, h * half:(h + 1) * half],
                             x[b, (1 - h) * half:(2 - h) * half]], axis=0)
        in_maps.append({"x": np.ascontiguousarray(xr), **shared})
    return in_maps


def kernel(x, ln1_g, ln1_b, ln2_g, ln2_b, w_qkv, w_proj, b_proj,
           w_fc1, b_fc1, w_fc2, b_fc2, num_heads=16, block_size=16):
    x = np.asarray(x, np.float32)
    B, S, Dm = x.shape
    half = S // 2
    trivial_gb = (np.all(np.asarray(ln1_g) == 1) and np.all(np.asarray(ln2_g) == 1)
                  and np.all(np.asarray(ln1_b) == 0) and np.all(np.asarray(ln2_b) == 0))
    nc = _get_nc(half, S, apply_gb=not trivial_gb)
    in_maps = make_in_maps(x, ln1_g, ln1_b, ln2_g, ln2_b, w_qkv, w_proj, b_proj,
                           w_fc1, b_fc1, w_fc2, b_fc2)
    res = bass_utils.run_bass_kernel_spmd(nc, in_maps, core_ids=list(range(8)))
    out = np.empty((B, S, Dm), np.float32)
    for c in range(8):
        b, h = c // 2, c % 2
        out[b, h * half:(h + 1) * half] = res.results[c]["out"]
    return out


# revision 20
# speedup vs baseline: 1.0532x; 1.0532x over previous
"""Trainium2 Bass kernel for nn_Block_59983513256170 (dense transformer block).

Sharding: 8 cores = (batch 4) x (sequence halves 2). Each core computes the
block for 512 query tokens of one batch element, redundantly computing
LN1+quant and K/V over that batch element's full 1024-token sequence so no
cross-core communication is needed. Host rotates each core's token order so
its own 512 tokens are always rows [0:512] (attention is permutation
invariant over keys), letting all cores run one identical SPMD program.

Precision: attention branch (qkv/scores/AV/proj) runs on fp32r matmuls
(~12 mantissa bits at bf16 speed) because its output feeds the second
block-FP quantization, where noise flips quantization bins. The MLP runs in
bf16 (its error does not cross a quantization boundary). LN, BFP quant,
softmax arithmetic and residuals are fp32. The BFP quantization uses an
exact bit-trick: scale = pow2(amax) via mantissa masking, round-to-nearest-
even via the +/- 1.5*2^23*scale addition trick, then clamp to +/-7*scale.
"""

import sys

sys.path.insert(0, "/opt/trn_rl_repo")

import numpy as np
import ml_dtypes

import concourse.bass as bass
import concourse.bacc as bacc
import concourse.tile as tile
import concourse.mybir as mybir
from concourse import bass_utils
from concourse.masks import make_identity

F32 = mybir.dt.float32
F32R = mybir.dt.float32r
BF16 = mybir.dt.bfloat16
AF = mybir.ActivationFunctionType
OP = mybir.AluOpType

D = 1024
H = 16
DH = 64
DFF = 4096
LN_EPS = 1e-6


def bcast16(t):
    """View a [128, nb] tile as [128, nb, 16] with the last dim broadcast."""
    ap = [list(x) for x in t.ap]
    return bass.AP(tensor=t.tensor, offset=t.offset, ap=ap + [[0, 16]])


def build_nc(Tq, Tkv, apply_gb=True):
    """Build the per-core Bass program. Tq = own query tokens, Tkv = full
    sequence tokens of this core's batch element (own tokens first)."""
    nq = Tq // 128   # query token tiles
    nk = Tkv // 128  # kv token tiles
    nc = bacc.Bacc("TRN2", target_bir_lowering=False, debug=False)

    x_d = nc.dram_tensor("x", [Tkv, D], F32, kind="ExternalInput").ap()
    wqkv_d = nc.dram_tensor("w_qkv", [D, 3 * D], F32R, kind="ExternalInput").ap()
    wproj_d = nc.dram_tensor("w_proj", [D, D], F32R, kind="ExternalInput").ap()
    bproj_d = nc.dram_tensor("b_proj", [D], F32, kind="ExternalInput").ap()
    wfc1_d = nc.dram_tensor("w_fc1", [D, DFF], BF16, kind="ExternalInput").ap()
    bfc1_d = nc.dram_tensor("b_fc1", [DFF], F32, kind="ExternalInput").ap()
    wfc2_d = nc.dram_tensor("w_fc2", [DFF, D], BF16, kind="ExternalInput").ap()
    bfc2_d = nc.dram_tensor("b_fc2", [D], F32, kind="ExternalInput").ap()
    g1_d = nc.dram_tensor("ln1_g", [D], F32, kind="ExternalInput").ap()
    b1_d = nc.dram_tensor("ln1_b", [D], F32, kind="ExternalInput").ap()
    g2_d = nc.dram_tensor("ln2_g", [D], F32, kind="ExternalInput").ap()
    b2_d = nc.dram_tensor("ln2_b", [D], F32, kind="ExternalInput").ap()
    out_d = nc.dram_tensor("out", [Tq, D], F32, kind="ExternalOutput").ap()

    def vec_bcast(pool, dram_vec, name, dtype=F32):
        """DRAM [D] vector -> SBUF [128, D] broadcast tile."""
        t = pool.tile([128, dram_vec.shape[0]], dtype, name=name)
        src = bass.AP(tensor=dram_vec.tensor, offset=dram_vec.offset,
                      ap=[[0, 128]] + [list(x) for x in dram_vec.ap])
        nc.sync.dma_start(out=t, in_=src)
        return t

    with tile.TileContext(nc) as tc:
        _cms = {}

        def open_pool(name, bufs, space="SBUF"):
            cm = tc.tile_pool(name=name, bufs=bufs, space=space)
            _cms[name] = cm
            return cm.__enter__()

        def close_pool(name):
            _cms.pop(name).__exit__(None, None, None)

        consts = open_pool("consts", 1)
        psum = open_pool("psum", 7, space="PSUM")
        dummy_ps = open_pool("dummy_ps", 1, space="PSUM")
        resid = open_pool("resid", 1)
        small = open_pool("small", 3)
        h2Tp = open_pool("h2Tp", 1)
        h2p = open_pool("h2p", 2)
        h2qp = open_pool("h2qp", 4)
        attn_big = open_pool("attn_big", 1)

        ident = consts.tile([128, 128], BF16, name="ident")
        make_identity(nc, ident)
        eps_t = consts.tile([128, 1], F32, name="eps")
        nc.vector.memset(eps_t, LN_EPS)
        if apply_gb:
            g1b = vec_bcast(consts, g1_d, "g1b")
            b1b = vec_bcast(consts, b1_d, "b1b")
            g2b = vec_bcast(consts, g2_d, "g2b")
            b2b = vec_bcast(consts, b2_d, "b2b")
        else:
            g1b = b1b = g2b = b2b = None
        bpb = vec_bcast(consts, bproj_d, "bpb")
        bf2b = vec_bcast(consts, bfc2_d, "bf2b")
        # b_fc1 as per-partition bias columns: [128, 32], [p, c] = b_fc1[c*128+p]
        bfc1_sb = consts.tile([128, DFF // 128], F32, name="bfc1")
        nc.sync.dma_start(out=bfc1_sb, in_=bfc1_d.rearrange("(c p) -> p c", p=128))

        # persistent across attention: packed qT/kT/v65/o and residual stream
        qT = attn_big.tile([128, 8, Tq], F32R, name="qT")       # q feature-major
        kT = attn_big.tile([128, 8, Tkv], F32R, name="kT")      # k feature-major
        v65 = attn_big.tile([128, H, nk, 65], F32R, name="v65")  # v token-major + ones col
        o_p = attn_big.tile([128, 8, Tq], F32R, name="o_p")     # normalized attn out
        x2 = resid.tile([128, nq, D], F32, name="x2")        # x + b_proj, then attn residual

        idf32 = consts.tile([128, 128], F32, name="idf32")
        nc.vector.memset(idf32, 0.0)

        def warm_f32(dep):
            """Small PE touch dependent on a fp32 tile (as rhs): defeats the
            HAM idle-detector during DVE-bound stretches."""
            k = dep.shape[0]
            dp = dummy_ps.tile([128, 64], F32, name="dummy")
            nc.tensor.matmul(dp, idf32[0:k, :], dep[:, 0:64], start=True, stop=True)

        def warm_bf16(dep):
            dp = dummy_ps.tile([128, 64], F32, name="dummy")
            nc.tensor.matmul(dp, ident, dep[:, 0:64], start=True, stop=True)

        def layernorm_quant(xt, g_b, b_b, hpool, hname, qpool=None):
            """token-major [128, D] fp32 -> LN -> BFP quant -> bf16 tile."""
            st = small.tile([128, 2, 6], F32, name="bnst")
            nc.vector.bn_stats(out=st[:, 0, :], in_=xt[:, 0:512])
            nc.vector.bn_stats(out=st[:, 1, :], in_=xt[:, 512:1024])
            mv = small.tile([128, 2], F32, name="bnmv")
            nc.vector.bn_aggr(out=mv, in_=st)
            rs = small.tile([128, 1], F32, name="rs")
            nc.scalar.activation(rs, mv[:, 1:2], AF.Sqrt, bias=eps_t, scale=1.0)
            rr = small.tile([128, 1], F32, name="rr")
            nc.vector.reciprocal(rr, rs)
            ht = hpool.tile([128, D], F32, name=hname + "_f")
            nc.vector.tensor_scalar(out=ht, in0=xt, scalar1=mv[:, 0:1], scalar2=rr,
                                    op0=OP.subtract, op1=OP.mult)
            if g_b is not None:
                nc.gpsimd.tensor_tensor(out=ht, in0=ht, in1=g_b, op=OP.mult)
                nc.gpsimd.tensor_tensor(out=ht, in0=ht, in1=b_b, op=OP.add)
            # BFP quant: blocks of 16 along features
            nb = D // 16
            amax = small.tile([128, nb], F32, name="amax")
            nc.vector.tensor_reduce(amax, ht.rearrange("p (b k) -> p b k", k=16),
                                    axis=mybir.AxisListType.X, op=OP.max,
                                    apply_absolute_value=True)
            nc.vector.tensor_scalar(out=amax.bitcast(mybir.dt.uint32),
                                    in0=amax.bitcast(mybir.dt.uint32),
                                    scalar1=0xFF800000, scalar2=None,
                                    op0=OP.bitwise_and)
            cc = small.tile([128, nb], F32, name="cc")
            nc.vector.tensor_scalar_mul(cc, amax, float(1.5 * 2 ** 20))
            warm_f32(cc)
            hi = small.tile([128, nb], F32, name="hi")
            nc.vector.tensor_scalar_mul(hi, amax, 7.0 / 8.0)
            lo = small.tile([128, nb], F32, name="lo")
            nc.vector.tensor_scalar_mul(lo, amax, -7.0 / 8.0)
            h3 = ht.rearrange("p (b k) -> p b k", k=16)
            nc.vector.tensor_tensor(out=h3, in0=h3, in1=bcast16(cc), op=OP.add)
            nc.vector.tensor_tensor(out=h3, in0=h3, in1=bcast16(cc), op=OP.subtract)
            nc.vector.tensor_tensor(out=h3, in0=h3, in1=bcast16(hi), op=OP.min)
            hq = (qpool or hpool).tile([128, D], BF16, name=hname)
            nc.vector.tensor_tensor(out=hq.rearrange("p (b k) -> p b k", k=16),
                                    in0=h3, in1=bcast16(lo), op=OP.max)
            warm_bf16(hq)
            return hq

        # ---------------- phase 1+2: LN1 + quant + transpose ----------------
        h1fmp = open_pool("h1fmp", 1)
        h1p = open_pool("h1p", 2)
        h1_fm = h1fmp.tile([128, 8, Tkv], F32R, name="h1fm")  # feature-major quantized
        with nc.named_scope("ln1_quant"):
            for tt in range(nk):
                xt = h1p.tile([128, D], F32, name="xt")
                nc.sync.dma_start(out=xt, in_=x_d[tt * 128:(tt + 1) * 128, :])
                hq = layernorm_quant(xt, g1b, b1b, h1p, "h1tok")
                for dd in range(8):
                    pst = psum.tile([128, 128], BF16, name="ps")
                    nc.tensor.transpose(pst, hq[:, dd * 128:(dd + 1) * 128], ident)
                    nc.scalar.copy(h1_fm[:, dd, tt * 128:(tt + 1) * 128], pst)
        close_pool("h1p")

        # ---------------- phase 3a: q, k (feature-major) ----------------
        # q: w cols [0, 1024)  -> qT ; k: w cols [1024, 2048) -> kT
        wkp = open_pool("wkp", 7)
        with nc.named_scope("qk_mm"):
            for blk in range(8):  # 8 blocks of 2 e-chunks (4 q blocks, 4 k blocks)
                is_q = blk < 4
                toks = Tq if is_q else Tkv
                ntc = toks // 512 if toks >= 512 else 1
                ntok = min(toks, 512)
                col0 = blk * 256 if is_q else 1024 + (blk - 4) * 256
                pss = [psum.tile([128, ntok], F32, name="ps") for _ in range(2 * ntc)]
                for d in range(8):
                    wt = wkp.tile([128, 256], F32R, name="wqk")
                    eng = nc.sync if d % 2 == 0 else nc.gpsimd
                    eng.dma_start(out=wt,
                                  in_=wqkv_d[d * 128:(d + 1) * 128, col0:col0 + 256])
                    for e2 in range(2):
                        for th in range(ntc):
                            nc.tensor.matmul(pss[e2 * ntc + th],
                                             wt[:, e2 * 128:(e2 + 1) * 128],
                                             h1_fm[:, d, th * 512:th * 512 + ntok],
                                             start=(d == 0), stop=(d == 7))
                dst = qT if is_q else kT
                ec0 = blk * 2 if is_q else (blk - 4) * 2
                for e2 in range(2):
                    for th in range(ntc):
                        nc.scalar.copy(
                            dst[:, ec0 + e2, th * 512:th * 512 + ntok],
                            pss[e2 * ntc + th])

        close_pool("wkp")

        # ---------------- phase 3b: v (token-major, + ones col) ----------------
        with nc.named_scope("v_mm"):
            for vc in range(2):
                wvp = open_pool("wvp", 1)
                wv = wvp.tile([128, 8, 512], F32R, name="wv")
                for d in range(8):
                    eng = nc.sync if d % 2 == 0 else nc.gpsimd
                    eng.dma_start(
                        out=wv[:, d, :],
                        in_=wqkv_d[d * 128:(d + 1) * 128,
                                   2048 + vc * 512:2048 + (vc + 1) * 512])
                for tch in range(nk):
                    ps = psum.tile([128, 512], F32, name="ps")
                    for d in range(8):
                        nc.tensor.matmul(ps, h1_fm[:, d, tch * 128:(tch + 1) * 128],
                                         wv[:, d, :], start=(d == 0), stop=(d == 7))
                    for hh in range(8):
                        head = vc * 8 + hh
                        nc.scalar.copy(v65[:, head, tch, 0:64],
                                       ps[:, hh * 64:(hh + 1) * 64])
                        nc.gpsimd.memset(v65[:, head, tch, 64:65].bitcast(F32), 1.0)
                close_pool("wvp")
        close_pool("h1fmp")

        # ---------------- phase 4: attention per head pair ----------------
        atp = open_pool("atp", 3)
        onp = open_pool("onp", 2)
        wpp = open_pool("wpp", 1)
        wproj_sb = wpp.tile([128, 8, D], F32R, name="wproj")
        for d in range(8):
            eng = nc.sync if d % 2 == 0 else nc.gpsimd
            eng.dma_start(out=wproj_sb[:, d, :],
                          in_=wproj_d[d * 128:(d + 1) * 128, :])
        # x + b_proj staged into x2 while attention runs
        for tcq in range(nq):
            nc.sync.dma_start(out=x2[:, tcq, :], in_=x_d[tcq * 128:(tcq + 1) * 128, :])
            nc.vector.tensor_tensor(out=x2[:, tcq, :], in0=x2[:, tcq, :], in1=bpb,
                                    op=OP.add)

        with nc.named_scope("attn"):
            for j in range(8):  # head pair (2j, 2j+1)
                ps_o = [psum.tile([65, Tq], F32, name="ps") for _ in range(2)]
                for kc in range(nk):
                    for ab in range(2):
                        ps_s = psum.tile([128, Tq], F32, name="ps")
                        nc.tensor.matmul(
                            ps_s,
                            kT[ab * 64:(ab + 1) * 64, j, kc * 128:(kc + 1) * 128],
                            qT[ab * 64:(ab + 1) * 64, j, :],
                            start=True, stop=True)
                        ee = atp.tile([128, Tq], F32R, name="expT")
                        nc.scalar.activation(ee, ps_s, AF.Exp, scale=0.125)
                        nc.tensor.matmul(ps_o[ab], v65[:, 2 * j + ab, kc, :], ee,
                                         start=(kc == 0), stop=(kc == nk - 1))
                for ab in range(2):
                    osb = onp.tile([65, Tq], F32, name="osb")
                    nc.scalar.copy(osb, ps_o[ab])
                    if ab == 0:
                        warm_f32(osb[0:64, 0:64])
                    row = onp.tile([1, Tq], F32, name="row")
                    nc.sync.dma_start(out=row, in_=osb[64:65, :])
                    rrow = onp.tile([1, Tq], F32, name="rrow")
                    nc.vector.reciprocal_approx_fast(rrow, row)
                    r64 = onp.tile([64, Tq], F32, name="r64")
                    nc.gpsimd.partition_broadcast(r64, rrow)
                    if ab == 0:
                        nc.vector.tensor_tensor(out=o_p[0:64, j, :],
                                                in0=osb[0:64, :], in1=r64,
                                                op=OP.mult)
                    else:
                        ob = onp.tile([64, Tq], F32R, name="ob")
                        nc.vector.tensor_tensor(out=ob, in0=osb[0:64, :],
                                                in1=r64, op=OP.mult)
                        nc.sync.dma_start(out=o_p[64:128, j, :], in_=ob)

        # ---------------- phase 5: proj + residual ----------------
        h2T = h2Tp.tile([128, 8, Tq], BF16, name="h2T")
        hq2s = []
        with nc.named_scope("proj_ln2"):
            for tcq in range(nq):
                for nn in range(2):
                    ps = psum.tile([128, 512], F32, name="ps")
                    for j in range(8):
                        nc.tensor.matmul(ps, o_p[:, j, tcq * 128:(tcq + 1) * 128],
                                         wproj_sb[:, j, nn * 512:(nn + 1) * 512],
                                         start=(j == 0), stop=(j == 7))
                    nc.vector.tensor_tensor(out=x2[:, tcq, nn * 512:(nn + 1) * 512],
                                            in0=ps,
                                            in1=x2[:, tcq, nn * 512:(nn + 1) * 512],
                                            op=OP.add)
                # LN2 + quant for this token tile right away (transposes are
                # hoisted below the loop: the in-order PE queue would stall
                # proj(tc+1) MMs behind transposes waiting on the DVE chain)
                hq = layernorm_quant(x2[:, tcq, :], g2b, b2b, h2p, "h2tok",
                                     qpool=h2qp)
                hq2s.append(hq)
                # x2 += b_fc2 after LN2 consumed it (final residual base)
                nc.vector.tensor_tensor(out=x2[:, tcq, :], in0=x2[:, tcq, :],
                                        in1=bf2b, op=OP.add)
            for tcq in range(nq):
                for dd in range(8):
                    pst = psum.tile([128, 128], BF16, name="ps")
                    nc.tensor.transpose(pst, hq2s[tcq][:, dd * 128:(dd + 1) * 128],
                                        ident)
                    nc.scalar.copy(h2T[:, dd, tcq * 128:(tcq + 1) * 128], pst)
        close_pool("wpp")
        close_pool("onp")
        close_pool("atp")
        close_pool("attn_big")

        # ---------------- phase 7 prep ----------------
        mlp = open_pool("mlp", 1)
        mT = mlp.tile([128, DFF // 128, Tq], BF16, name="mT")
        wfc2_sb = mlp.tile([128, DFF // 128, D], BF16, name="wfc2")
        for g in range(8):
            eng = nc.sync if g % 2 == 0 else nc.gpsimd
            eng.dma_start(
                out=wfc2_sb[:, g * 4:(g + 1) * 4, :],
                in_=wfc2_d.rearrange("(c p) n -> p c n", p=128)[:, g * 4:(g + 1) * 4, :])

        # ---------------- phase 7: fc1 + gelu (feature-major m) ----------------
        wf1p = open_pool("wf1p", 6)
        wfc1_r = wfc1_d.rearrange("(c p) n -> p c n", p=128)
        with nc.named_scope("fc1"):
            for hc in range(DFF // 128):
                wt = wf1p.tile([128, 8, 128], BF16, name="wfc1")
                eng = nc.sync if hc % 2 == 0 else nc.gpsimd
                eng.dma_start(out=wt, in_=wfc1_r[:, :, hc * 128:(hc + 1) * 128])
                ps = psum.tile([128, Tq], F32, name="ps")
                for d in range(8):
                    nc.tensor.matmul(ps, wt[:, d, :], h2T[:, d, :],
                                     start=(d == 0), stop=(d == 7))
                nc.scalar.activation(mT[:, hc, :], ps, AF.Gelu,
                                     bias=bfc1_sb[:, hc:hc + 1], scale=1.0)

        # ---------------- phase 8: fc2 + residual -> out ----------------
        outp = open_pool("outp", 2)
        with nc.named_scope("fc2"):
            for tcq in range(nq):
                ot = outp.tile([128, D], F32, name="ot")
                for nn in range(2):
                    ps = psum.tile([128, 512], F32, name="ps")
                    for hc in range(DFF // 128):
                        nc.tensor.matmul(ps, mT[:, hc, tcq * 128:(tcq + 1) * 128],
                                         wfc2_sb[:, hc, nn * 512:(nn + 1) * 512],
                                         start=(hc == 0), stop=(hc == DFF // 128 - 1))
                    nc.vector.tensor_tensor(out=ot[:, nn * 512:(nn + 1) * 512],
                                            in0=ps,
                                            in1=x2[:, tcq, nn * 512:(nn + 1) * 512],
                                            op=OP.add)
                nc.sync.dma_start(out=out_d[tcq * 128:(tcq + 1) * 128, :], in_=ot)

        close_pool("outp")
        close_pool("wf1p")
        close_pool("mlp")
        close_pool("h2qp")
        close_pool("h2p")
        close_pool("h2Tp")
        close_pool("small")
        close_pool("resid")
        close_pool("dummy_ps")
        close_pool("psum")
        close_pool("consts")

    nc.finalize()
    return nc


_NC_CACHE = {}


def _get_nc(Tq, Tkv, apply_gb=True):
    key = (Tq, Tkv, apply_gb)
    if key not in _NC_CACHE:
        _NC_CACHE[key] = build_nc(Tq, Tkv, apply_gb)
    return _NC_CACHE[key]


def make_in_maps(x, ln1_g, ln1_b, ln2_g, ln2_b, w_qkv, w_proj, b_proj,
                 w_fc1, b_fc1, w_fc2, b_fc2, n_cores=8):
    x = np.asarray(x, np.float32)
    B, S, _ = x.shape
    half = S // 2
    shared = {
        "w_qkv": np.ascontiguousarray(np.asarray(w_qkv, np.float32)),
        "w_proj": np.ascontiguousarray(np.asarray(w_proj, np.float32)),
        "b_proj": np.asarray(b_proj, np.float32),
        "w_fc1": np.ascontiguousarray(np.asarray(w_fc1, np.float32).astype(ml_dtypes.bfloat16)),
        "b_fc1": np.asarray(b_fc1, np.float32),
        "w_fc2": np.ascontiguousarray(np.asarray(w_fc2, np.float32).astype(ml_dtypes.bfloat16)),
        "b_fc2": np.asarray(b_fc2, np.float32),
        "ln1_g": np.asarray(ln1_g, np.float32),
        "ln1_b": np.asarray(ln1_b, np.float32),
        "ln2_g": np.asarray(ln2_g, np.float32),
        "ln2_b": np.asarray(ln2_b, np.float32),
    }
    in_maps = []
    for c in range(n_cores):
        b, h = c // 2, c % 2
        xr = np.concatenate([x[b, h * half:(h + 1) * half],
                             x[b, (1 - h) * half:(2 - h) * half]], axis=0)
        in_maps.append({"x": np.ascontiguousarray(xr), **shared})
    return in_maps


def kernel(x, ln1_g, ln1_b, ln2_g, ln2_b, w_qkv, w_proj, b_proj,
           w_fc1, b_fc1, w_fc2, b_fc2, num_heads=16, block_size=16):
    x = np.asarray(x, np.float32)
    B, S, Dm = x.shape
    half = S // 2
    trivial_gb = (np.all(np.asarray(ln1_g) == 1) and np.all(np.asarray(ln2_g) == 1)
                  and np.all(np.asarray(ln1_b) == 0) and np.all(np.asarray(ln2_b) == 0))
    nc = _get_nc(half, S, apply_gb=not trivial_gb)
    in_maps = make_in_maps(x, ln1_g, ln1_b, ln2_g, ln2_b, w_qkv, w_proj, b_proj,
                           w_fc1, b_fc1, w_fc2, b_fc2)
    res = bass_utils.run_bass_kernel_spmd(nc, in_maps, core_ids=list(range(8)))
    out = np.empty((B, S, Dm), np.float32)
    for c in range(8):
        b, h = c // 2, c % 2
        out[b, h * half:(h + 1) * half] = res.results[c]["out"]
    return out


# revision 23
# speedup vs baseline: 1.1003x; 1.0447x over previous
"""Trainium2 Bass kernel for nn_Block_59983513256170 (dense transformer block).

Sharding: 8 cores = (batch 4) x (sequence halves 2). Each core computes the
block for 512 query tokens of one batch element, redundantly computing
LN1+quant and K/V over that batch element's full 1024-token sequence so no
cross-core communication is needed. Host rotates each core's token order so
its own 512 tokens are always rows [0:512] (attention is permutation
invariant over keys), letting all cores run one identical SPMD program.

Precision: attention branch (qkv/scores/AV/proj) runs on fp32r matmuls
(~12 mantissa bits at bf16 speed) because its output feeds the second
block-FP quantization, where noise flips quantization bins. The MLP runs in
bf16 (its error does not cross a quantization boundary). LN, BFP quant,
softmax arithmetic and residuals are fp32. The BFP quantization uses an
exact bit-trick: scale = pow2(amax) via mantissa masking, round-to-nearest-
even via the +/- 1.5*2^23*scale addition trick, then clamp to +/-7*scale.
"""

import sys

sys.path.insert(0, "/opt/trn_rl_repo")

import numpy as np
import ml_dtypes

import concourse.bass as bass
import concourse.bacc as bacc
import concourse.tile as tile
import concourse.mybir as mybir
from concourse import bass_utils
from concourse.masks import make_identity

F32 = mybir.dt.float32
F32R = mybir.dt.float32r
BF16 = mybir.dt.bfloat16
AF = mybir.ActivationFunctionType
OP = mybir.AluOpType

D = 1024
H = 16
DH = 64
DFF = 4096
LN_EPS = 1e-6


def bcast16(t):
    """View a [128, nb] tile as [128, nb, 16] with the last dim broadcast."""
    ap = [list(x) for x in t.ap]
    return bass.AP(tensor=t.tensor, offset=t.offset, ap=ap + [[0, 16]])


def build_nc(Tq, Tkv, apply_gb=True):
    """Build the per-core Bass program. Tq = own query tokens, Tkv = full
    sequence tokens of this core's batch element (own tokens first)."""
    nq = Tq // 128   # query token tiles
    nk = Tkv // 128  # kv token tiles
    nc = bacc.Bacc("TRN2", target_bir_lowering=False, debug=False)

    x_d = nc.dram_tensor("x", [Tkv, D], F32, kind="ExternalInput").ap()
    wqkv_d = nc.dram_tensor("w_qkv", [D, 3 * D], F32R, kind="ExternalInput").ap()
    wproj_d = nc.dram_tensor("w_proj", [D, D], F32R, kind="ExternalInput").ap()
    bproj_d = nc.dram_tensor("b_proj", [D], F32, kind="ExternalInput").ap()
    wfc1_d = nc.dram_tensor("w_fc1", [D, DFF], BF16, kind="ExternalInput").ap()
    bfc1_d = nc.dram_tensor("b_fc1", [DFF], F32, kind="ExternalInput").ap()
    wfc2_d = nc.dram_tensor("w_fc2", [DFF, D], BF16, kind="ExternalInput").ap()
    bfc2_d = nc.dram_tensor("b_fc2", [D], F32, kind="ExternalInput").ap()
    g1_d = nc.dram_tensor("ln1_g", [D], F32, kind="ExternalInput").ap()
    b1_d = nc.dram_tensor("ln1_b", [D], F32, kind="ExternalInput").ap()
    g2_d = nc.dram_tensor("ln2_g", [D], F32, kind="ExternalInput").ap()
    b2_d = nc.dram_tensor("ln2_b", [D], F32, kind="ExternalInput").ap()
    out_d = nc.dram_tensor("out", [Tq, D], F32, kind="ExternalOutput").ap()

    def vec_bcast(pool, dram_vec, name, dtype=F32):
        """DRAM [D] vector -> SBUF [128, D] broadcast tile."""
        t = pool.tile([128, dram_vec.shape[0]], dtype, name=name)
        src = bass.AP(tensor=dram_vec.tensor, offset=dram_vec.offset,
                      ap=[[0, 128]] + [list(x) for x in dram_vec.ap])
        nc.sync.dma_start(out=t, in_=src)
        return t

    with tile.TileContext(nc) as tc:
        _cms = {}

        def open_pool(name, bufs, space="SBUF"):
            cm = tc.tile_pool(name=name, bufs=bufs, space=space)
            _cms[name] = cm
            return cm.__enter__()

        def close_pool(name):
            _cms.pop(name).__exit__(None, None, None)

        consts = open_pool("consts", 1)
        psum = open_pool("psum", 7, space="PSUM")
        dummy_ps = open_pool("dummy_ps", 1, space="PSUM")
        resid = open_pool("resid", 1)
        small = open_pool("small", 3)
        h2Tp = open_pool("h2Tp", 1)
        h2p = open_pool("h2p", 2)
        h2qp = open_pool("h2qp", 4)
        attn_big = open_pool("attn_big", 1)

        ident = consts.tile([128, 128], BF16, name="ident")
        make_identity(nc, ident)
        eps_t = consts.tile([128, 1], F32, name="eps")
        nc.vector.memset(eps_t, LN_EPS)
        if apply_gb:
            g1b = vec_bcast(consts, g1_d, "g1b")
            b1b = vec_bcast(consts, b1_d, "b1b")
            g2b = vec_bcast(consts, g2_d, "g2b")
            b2b = vec_bcast(consts, b2_d, "b2b")
        else:
            g1b = b1b = g2b = b2b = None
        bpb = vec_bcast(consts, bproj_d, "bpb")
        bf2b = vec_bcast(consts, bfc2_d, "bf2b")
        # b_fc1 as per-partition bias columns: [128, 32], [p, c] = b_fc1[c*128+p]
        bfc1_sb = consts.tile([128, DFF // 128], F32, name="bfc1")
        nc.sync.dma_start(out=bfc1_sb, in_=bfc1_d.rearrange("(c p) -> p c", p=128))

        # persistent across attention: packed qT/kT/v65/o and residual stream
        qT = attn_big.tile([128, 8, Tq], F32R, name="qT")       # q feature-major
        kT = attn_big.tile([128, 8, Tkv], F32R, name="kT")      # k feature-major
        v65 = attn_big.tile([128, H, nk, 65], F32R, name="v65")  # v token-major + ones col
        o_p = attn_big.tile([128, 8, Tq], F32R, name="o_p")     # normalized attn out
        x2 = resid.tile([128, nq, D], F32, name="x2")        # x + b_proj, then attn residual

        idf32 = consts.tile([128, 128], F32, name="idf32")
        nc.vector.memset(idf32, 0.0)

        def warm_f32(dep):
            """Small PE touch dependent on a fp32 tile (as rhs): defeats the
            HAM idle-detector during DVE-bound stretches."""
            k = dep.shape[0]
            dp = dummy_ps.tile([128, 64], F32, name="dummy")
            nc.tensor.matmul(dp, idf32[0:k, :], dep[:, 0:64], start=True, stop=True)

        def warm_bf16(dep):
            dp = dummy_ps.tile([128, 64], F32, name="dummy")
            nc.tensor.matmul(dp, ident, dep[:, 0:64], start=True, stop=True)

        def layernorm_quant(xt, g_b, b_b, hpool, hname, qpool=None):
            """token-major [128, D] fp32 -> LN -> BFP quant -> bf16 tile."""
            st = small.tile([128, 2, 6], F32, name="bnst")
            nc.vector.bn_stats(out=st[:, 0, :], in_=xt[:, 0:512])
            nc.vector.bn_stats(out=st[:, 1, :], in_=xt[:, 512:1024])
            mv = small.tile([128, 2], F32, name="bnmv")
            nc.vector.bn_aggr(out=mv, in_=st)
            rs = small.tile([128, 1], F32, name="rs")
            nc.scalar.activation(rs, mv[:, 1:2], AF.Sqrt, bias=eps_t, scale=1.0)
            rr = small.tile([128, 1], F32, name="rr")
            nc.vector.reciprocal(rr, rs)
            ht = hpool.tile([128, D], F32, name=hname + "_f")
            nc.vector.tensor_scalar(out=ht, in0=xt, scalar1=mv[:, 0:1], scalar2=rr,
                                    op0=OP.subtract, op1=OP.mult)
            if g_b is not None:
                nc.gpsimd.tensor_tensor(out=ht, in0=ht, in1=g_b, op=OP.mult)
                nc.gpsimd.tensor_tensor(out=ht, in0=ht, in1=b_b, op=OP.add)
            # BFP quant: blocks of 16 along features
            nb = D // 16
            amax = small.tile([128, nb], F32, name="amax")
            nc.vector.tensor_reduce(amax, ht.rearrange("p (b k) -> p b k", k=16),
                                    axis=mybir.AxisListType.X, op=OP.max,
                                    apply_absolute_value=True)
            nc.vector.tensor_scalar(out=amax.bitcast(mybir.dt.uint32),
                                    in0=amax.bitcast(mybir.dt.uint32),
                                    scalar1=0xFF800000, scalar2=None,
                                    op0=OP.bitwise_and)
            cc = small.tile([128, nb], F32, name="cc")
            nc.vector.tensor_scalar_mul(cc, amax, float(1.5 * 2 ** 20))
            warm_f32(cc)
            hi = small.tile([128, nb], F32, name="hi")
            nc.vector.tensor_scalar_mul(hi, amax, 7.0 / 8.0)
            lo = small.tile([128, nb], F32, name="lo")
            nc.vector.tensor_scalar_mul(lo, amax, -7.0 / 8.0)
            h3 = ht.rearrange("p (b k) -> p b k", k=16)
            nc.vector.tensor_tensor(out=h3, in0=h3, in1=bcast16(cc), op=OP.add)
            nc.vector.tensor_tensor(out=h3, in0=h3, in1=bcast16(cc), op=OP.subtract)
            nc.vector.tensor_tensor(out=h3, in0=h3, in1=bcast16(hi), op=OP.min)
            hq = (qpool or hpool).tile([128, D], BF16, name=hname)
            nc.vector.tensor_tensor(out=hq.rearrange("p (b k) -> p b k", k=16),
                                    in0=h3, in1=bcast16(lo), op=OP.max)
            warm_bf16(hq)
            return hq

        # ---------------- phase 1+2: LN1 + quant + transpose ----------------
        h1fmp = open_pool("h1fmp", 1)
        h1p = open_pool("h1p", 2)
        h1_fm = h1fmp.tile([128, 8, Tkv], F32R, name="h1fm")  # feature-major quantized
        with nc.named_scope("ln1_quant"):
            for tt in range(nk):
                xt = h1p.tile([128, D], F32, name="xt")
                nc.sync.dma_start(out=xt, in_=x_d[tt * 128:(tt + 1) * 128, :])
                hq = layernorm_quant(xt, g1b, b1b, h1p, "h1tok")
                for dd in range(8):
                    pst = psum.tile([128, 128], BF16, name="ps")
                    nc.tensor.transpose(pst, hq[:, dd * 128:(dd + 1) * 128], ident)
                    nc.scalar.copy(h1_fm[:, dd, tt * 128:(tt + 1) * 128], pst)
        close_pool("h1p")

        # ---------------- phase 3a: q, k (feature-major) ----------------
        # q: w cols [0, 1024)  -> qT ; k: w cols [1024, 2048) -> kT
        wkp = open_pool("wkp", 6)
        with nc.named_scope("qk_mm"):
            for blk in range(8):  # 8 blocks of 2 e-chunks (4 q blocks, 4 k blocks)
                is_q = blk < 4
                toks = Tq if is_q else Tkv
                ntc = toks // 512 if toks >= 512 else 1
                ntok = min(toks, 512)
                col0 = blk * 256 if is_q else 1024 + (blk - 4) * 256
                pss = [psum.tile([128, ntok], F32, name="ps") for _ in range(2 * ntc)]
                for d in range(8):
                    wt = wkp.tile([128, 256], F32R, name="wqk")
                    eng = nc.sync if d % 2 == 0 else nc.gpsimd
                    eng.dma_start(out=wt,
                                  in_=wqkv_d[d * 128:(d + 1) * 128, col0:col0 + 256])
                    for e2 in range(2):
                        for th in range(ntc):
                            nc.tensor.matmul(pss[e2 * ntc + th],
                                             wt[:, e2 * 128:(e2 + 1) * 128],
                                             h1_fm[:, d, th * 512:th * 512 + ntok],
                                             start=(d == 0), stop=(d == 7))
                dst = qT if is_q else kT
                ec0 = blk * 2 if is_q else (blk - 4) * 2
                for e2 in range(2):
                    for th in range(ntc):
                        nc.scalar.copy(
                            dst[:, ec0 + e2, th * 512:th * 512 + ntok],
                            pss[e2 * ntc + th])

        close_pool("wkp")

        # ---------------- phase 3b: v (token-major, + ones col) ----------------
        with nc.named_scope("v_mm"):
            for vc in range(2):
                wvp = open_pool("wvp", 1)
                wv = wvp.tile([128, 8, 512], F32R, name="wv")
                for d in range(8):
                    eng = nc.sync if d % 2 == 0 else nc.gpsimd
                    eng.dma_start(
                        out=wv[:, d, :],
                        in_=wqkv_d[d * 128:(d + 1) * 128,
                                   2048 + vc * 512:2048 + (vc + 1) * 512])
                for tch in range(nk):
                    ps = psum.tile([128, 512], F32, name="ps")
                    for d in range(8):
                        nc.tensor.matmul(ps, h1_fm[:, d, tch * 128:(tch + 1) * 128],
                                         wv[:, d, :], start=(d == 0), stop=(d == 7))
                    for hh in range(8):
                        head = vc * 8 + hh
                        nc.scalar.copy(v65[:, head, tch, 0:64],
                                       ps[:, hh * 64:(hh + 1) * 64])
                        nc.gpsimd.memset(v65[:, head, tch, 64:65].bitcast(F32), 1.0)
                close_pool("wvp")
        close_pool("h1fmp")

        # ---------------- phase 4: attention per head pair ----------------
        atp = open_pool("atp", 3)
        onp = open_pool("onp", 2)
        wpp = open_pool("wpp", 1)
        wproj_sb = wpp.tile([128, 8, D], F32R, name="wproj")
        for d in range(8):
            eng = nc.sync if d % 2 == 0 else nc.gpsimd
            eng.dma_start(out=wproj_sb[:, d, :],
                          in_=wproj_d[d * 128:(d + 1) * 128, :])
        # x + b_proj staged into x2 while attention runs
        for tcq in range(nq):
            nc.sync.dma_start(out=x2[:, tcq, :], in_=x_d[tcq * 128:(tcq + 1) * 128, :])
            nc.vector.tensor_tensor(out=x2[:, tcq, :], in0=x2[:, tcq, :], in1=bpb,
                                    op=OP.add)

        with nc.named_scope("attn"):
            for j in range(8):  # head pair (2j, 2j+1)
                ps_o = [psum.tile([65, Tq], F32, name="ps") for _ in range(2)]
                for kc in range(nk):
                    for ab in range(2):
                        ps_s = psum.tile([128, Tq], F32, name="ps")
                        nc.tensor.matmul(
                            ps_s,
                            kT[ab * 64:(ab + 1) * 64, j, kc * 128:(kc + 1) * 128],
                            qT[ab * 64:(ab + 1) * 64, j, :],
                            start=True, stop=True)
                        ee = atp.tile([128, Tq], F32R, name="expT")
                        nc.scalar.activation(ee, ps_s, AF.Exp, scale=0.125)
                        nc.tensor.matmul(ps_o[ab], v65[:, 2 * j + ab, kc, :], ee,
                                         start=(kc == 0), stop=(kc == nk - 1))
                for ab in range(2):
                    osb = onp.tile([65, Tq], F32, name="osb")
                    nc.scalar.copy(osb, ps_o[ab])
                    if ab == 0:
                        warm_f32(osb[0:64, 0:64])
                    row = onp.tile([1, Tq], F32, name="row")
                    nc.sync.dma_start(out=row, in_=osb[64:65, :])
                    rrow = onp.tile([1, Tq], F32, name="rrow")
                    nc.vector.reciprocal_approx_fast(rrow, row)
                    r64 = onp.tile([64, Tq], F32, name="r64")
                    nc.gpsimd.partition_broadcast(r64, rrow)
                    if ab == 0:
                        nc.vector.tensor_tensor(out=o_p[0:64, j, :],
                                                in0=osb[0:64, :], in1=r64,
                                                op=OP.mult)
                    else:
                        ob = onp.tile([64, Tq], F32R, name="ob")
                        nc.vector.tensor_tensor(out=ob, in0=osb[0:64, :],
                                                in1=r64, op=OP.mult)
                        nc.sync.dma_start(out=o_p[64:128, j, :], in_=ob)

        # ---------------- phase 5: proj + residual ----------------
        h2T = h2Tp.tile([128, 8, Tq], BF16, name="h2T")
        hq2s = []
        with nc.named_scope("proj_ln2"):
            for tcq in range(nq):
                for nn in range(2):
                    ps = psum.tile([128, 512], F32, name="ps")
                    for j in range(8):
                        nc.tensor.matmul(ps, o_p[:, j, tcq * 128:(tcq + 1) * 128],
                                         wproj_sb[:, j, nn * 512:(nn + 1) * 512],
                                         start=(j == 0), stop=(j == 7))
                    nc.vector.tensor_tensor(out=x2[:, tcq, nn * 512:(nn + 1) * 512],
                                            in0=ps,
                                            in1=x2[:, tcq, nn * 512:(nn + 1) * 512],
                                            op=OP.add)
                # LN2 + quant for this token tile right away (transposes are
                # hoisted below the loop: the in-order PE queue would stall
                # proj(tc+1) MMs behind transposes waiting on the DVE chain)
                hq = layernorm_quant(x2[:, tcq, :], g2b, b2b, h2p, "h2tok",
                                     qpool=h2qp)
                hq2s.append(hq)
                # x2 += b_fc2 after LN2 consumed it (final residual base)
                nc.vector.tensor_tensor(out=x2[:, tcq, :], in0=x2[:, tcq, :],
                                        in1=bf2b, op=OP.add)
            for tcq in range(nq):
                for dd in range(8):
                    pst = psum.tile([128, 128], BF16, name="ps")
                    nc.tensor.transpose(pst, hq2s[tcq][:, dd * 128:(dd + 1) * 128],
                                        ident)
                    nc.scalar.copy(h2T[:, dd, tcq * 128:(tcq + 1) * 128], pst)
        close_pool("wpp")
        close_pool("onp")
        close_pool("atp")
        close_pool("attn_big")

        # ---------------- phase 7 prep ----------------
        mlp = open_pool("mlp", 1)
        mT = mlp.tile([128, DFF // 128, Tq], BF16, name="mT")
        wfc2_sb = mlp.tile([128, DFF // 128, D], BF16, name="wfc2")
        for g in range(8):
            eng = nc.sync if g % 2 == 0 else nc.gpsimd
            eng.dma_start(
                out=wfc2_sb[:, g * 4:(g + 1) * 4, :],
                in_=wfc2_d.rearrange("(c p) n -> p c n", p=128)[:, g * 4:(g + 1) * 4, :])

        # ---------------- phase 7: fc1 + gelu (feature-major m) ----------------
        wf1p = open_pool("wf1p", 6)
        wfc1_r = wfc1_d.rearrange("(c p) n -> p c n", p=128)
        with nc.named_scope("fc1"):
            for hc in range(DFF // 128):
                wt = wf1p.tile([128, 8, 128], BF16, name="wfc1")
                eng = nc.sync if hc % 2 == 0 else nc.gpsimd
                eng.dma_start(out=wt, in_=wfc1_r[:, :, hc * 128:(hc + 1) * 128])
                ps = psum.tile([128, Tq], F32, name="ps")
                for d in range(8):
                    nc.tensor.matmul(ps, wt[:, d, :], h2T[:, d, :],
                                     start=(d == 0), stop=(d == 7))
                nc.scalar.activation(mT[:, hc, :], ps, AF.Gelu,
                                     bias=bfc1_sb[:, hc:hc + 1], scale=1.0)

        # ---------------- phase 8: fc2 + residual -> out ----------------
        outp = open_pool("outp", 2)
        with nc.named_scope("fc2"):
            for tcq in range(nq):
                ot = outp.tile([128, D], F32, name="ot")
                for nn in range(2):
                    ps = psum.tile([128, 512], F32, name="ps")
                    for hc in range(DFF // 128):
                        nc.tensor.matmul(ps, mT[:, hc, tcq * 128:(tcq + 1) * 128],
                                         wfc2_sb[:, hc, nn * 512:(nn + 1) * 512],
                                         start=(hc == 0), stop=(hc == DFF // 128 - 1))
                    nc.vector.tensor_tensor(out=ot[:, nn * 512:(nn + 1) * 512],
                                            in0=ps,
                                            in1=x2[:, tcq, nn * 512:(nn + 1) * 512],
                                            op=OP.add)
                nc.sync.dma_start(out=out_d[tcq * 128:(tcq + 1) * 128, :], in_=ot)

        close_pool("outp")
        close_pool("wf1p")
        close_pool("mlp")
        close_pool("h2qp")
        close_pool("h2p")
        close_pool("h2Tp")
        close_pool("small")
        close_pool("resid")
        close_pool("dummy_ps")
        close_pool("psum")
        close_pool("consts")

    nc.finalize()
    return nc


_NC_CACHE = {}


def _get_nc(Tq, Tkv, apply_gb=True):
    key = (Tq, Tkv, apply_gb)
    if key not in _NC_CACHE:
        _NC_CACHE[key] = build_nc(Tq, Tkv, apply_gb)
    return _NC_CACHE[key]


def make_in_maps(x, ln1_g, ln1_b, ln2_g, ln2_b, w_qkv, w_proj, b_proj,
                 w_fc1, b_fc1, w_fc2, b_fc2, n_cores=8):
    x = np.asarray(x, np.float32)
    B, S, _ = x.shape
    half = S // 2
    shared = {
        "w_qkv": np.ascontiguousarray(np.asarray(w_qkv, np.float32)),
        "w_proj": np.ascontiguousarray(np.asarray(w_proj, np.float32)),
        "b_proj": np.asarray(b_proj, np.float32),
        "w_fc1": np.ascontiguousarray(np.asarray(w_fc1, np.float32).astype(ml_dtypes.bfloat16)),
        "b_fc1": np.asarray(b_fc1, np.float32),
        "w_fc2": np.ascontiguousarray(np.asarray(w_fc2, np.float32).astype(ml_dtypes.bfloat16)),
        "b_fc2": np.asarray(b_fc2, np.float32),
        "ln1_g": np.asarray(ln1_g, np.float32),
        "ln1_b": np.asarray(ln1_b, np.float32),
        "ln2_g": np.asarray(ln2_g, np.float32),
        "ln2_b": np.asarray(ln2_b, np.float32),
    }
    in_maps = []
    for c in range(n_cores):
        b, h = c // 2, c % 2
        xr = np.concatenate([x[b, h * half:(h + 1) * half],
                             x[b, (1 - h) * half:(2 - h) * half]], axis=0)
        in_maps.append({"x": np.ascontiguousarray(xr), **shared})
    return in_maps


def kernel(x, ln1_g, ln1_b, ln2_g, ln2_b, w_qkv, w_proj, b_proj,
           w_fc1, b_fc1, w_fc2, b_fc2, num_heads=16, block_size=16):
    x = np.asarray(x, np.float32)
    B, S, Dm = x.shape
    half = S // 2
    trivial_gb = (np.all(np.asarray(ln1_g) == 1) and np.all(np.asarray(ln2_g) == 1)
                  and np.all(np.asarray(ln1_b) == 0) and np.all(np.asarray(ln2_b) == 0))
    nc = _get_nc(half, S, apply_gb=not trivial_gb)
    in_maps = make_in_maps(x, ln1_g, ln1_b, ln2_g, ln2_b, w_qkv, w_proj, b_proj,
                           w_fc1, b_fc1, w_fc2, b_fc2)
    res = bass_utils.run_bass_kernel_spmd(nc, in_maps, core_ids=list(range(8)))
    out = np.empty((B, S, Dm), np.float32)
    for c in range(8):
        b, h = c // 2, c % 2
        out[b, h * half:(h + 1) * half] = res.results[c]["out"]
    return out
